# Initial kernel scaffold
#
"""Trainium2 Bass kernel for LogicGatedSNN.

Computes: spikes = (spike_input @ ternarize(synapse_states).T >= 1.0)
  where ternarize(s) = +1 if s > 1, -1 if s < -1, else 0.

Strategy:
  - Data-parallel over the batch dim across 8 NeuronCores (1024 rows/core),
    weights replicated.
  - On each core: ternarize W on DVE into bf16 {-1,0,+1} (exact),
    split X into bf16 hi/lo planes (x = hi + lo, residual ~2^-18 * x),
    round-trip both through DRAM scratch and reload with xbar transpose-DMA
    to get the contraction dim (k) onto SBUF partitions.
  - Matmul: psum[b_tile 128, j 512] accumulated over k (32 steps x hi/lo),
    stationary = X^T plane tiles [128k, 128b], moving = W'^T [128k, 512j].
  - Threshold (>= 1.0) on DVE straight out of PSUM, store natural [b, j].
"""

import sys

if "/opt/trn_rl_repo" not in sys.path:
    sys.path.insert(0, "/opt/trn_rl_repo")

import numpy as np

N_CORES = 8
BATCH, IN_F, OUT_F = 8192, 4096, 4096
B_CORE = BATCH // N_CORES  # 1024

_BUILT = None


def build_bass(B, K, J, JS=512, KCH=1024):
    """Build the per-core Bass program for x:[B,K] f32, w:[J,K] f32 -> out:[B,J] f32."""
    from concourse import bacc
    import concourse.mybir as mybir
    import concourse.tile as tile
    from concourse.bass import ts

    f32, bf16 = mybir.dt.float32, mybir.dt.bfloat16
    alu = mybir.AluOpType
    P = 128
    JS = min(JS, J)
    KCH = min(KCH, K)
    BH = min(512, B)          # batch rows per pass (<= 4 psum banks)
    NBP = B // BH             # number of batch passes
    BT = BH // P              # 128-row tiles per pass
    KT = K // P               # k tiles (partition-dim groups)
    NSLAB = J // JS           # output-feature slabs
    NKC = K // KCH            # staging chunks along k
    assert B % BH == 0 and BH % P == 0 and K % P == 0 and J % JS == 0

    nc = bacc.Bacc("TRN2", target_bir_lowering=False, debug=False)
    x = nc.dram_tensor("x", [B, K], f32, kind="ExternalInput")
    w = nc.dram_tensor("w", [J, K], f32, kind="ExternalInput")
    out = nc.dram_tensor("out", [B, J], f32, kind="ExternalOutput")

    with tile.TileContext(nc) as tc:
        with (
            tc.tile_pool(name="dram", bufs=1, space="DRAM") as dpool,
            tc.tile_pool(name="xstage32", bufs=2) as xs32,
            tc.tile_pool(name="xstage16", bufs=2) as xs16,
            tc.tile_pool(name="wstage32", bufs=2) as ws32,
            tc.tile_pool(name="wstage16", bufs=2) as ws16,
            tc.tile_pool(name="xtres", bufs=1) as xtres,
            tc.tile_pool(name="wtp", bufs=2) as wtp,
            tc.tile_pool(name="ostage", bufs=8) as op,
            tc.tile_pool(name="psum", bufs=8, space="PSUM") as pp,
        ):
            # DRAM scratch: ternarized W (natural layout), per j-slab for
            # slab-granular RAW pipelining; X hi/lo planes per batch pass.
            wt_nat = [
                dpool.tile([JS, K], bf16, name=f"wt_nat_s{s}") for s in range(NSLAB)
            ]
            xhi_nat = [
                dpool.tile([BH, K], bf16, name=f"xhi_nat_p{bp}") for bp in range(NBP)
            ]
            xlo_nat = [
                dpool.tile([BH, K], bf16, name=f"xlo_nat_p{bp}") for bp in range(NBP)
            ]

            for bp in range(NBP):
                # ---- X prep for this batch half: split f32 -> bf16 hi + lo ----
                for bsub in range(BT):
                    r0 = bp * BH + bsub * P
                    for kc in range(NKC):
                        c0 = kc * KCH
                        xin = xs32.tile([P, KCH], f32, name="xin")
                        nc.sync.dma_start(
                            out=xin[:], in_=x[r0 : r0 + P, c0 : c0 + KCH]
                        )
                        hi = xs16.tile([P, KCH], bf16, name="xhi")
                        nc.vector.tensor_copy(out=hi[:], in_=xin[:])
                        hi32 = xs32.tile([P, KCH], f32, name="xhi32")
                        nc.vector.tensor_copy(out=hi32[:], in_=hi[:])
                        lo = xs16.tile([P, KCH], bf16, name="xlo")
                        nc.vector.tensor_sub(out=lo[:], in0=xin[:], in1=hi32[:])
                        nc.sync.dma_start(
                            out=xhi_nat[bp][bsub * P : bsub * P + P, c0 : c0 + KCH],
                            in_=hi[:],
                        )
                        nc.sync.dma_start(
                            out=xlo_nat[bp][bsub * P : bsub * P + P, c0 : c0 + KCH],
                            in_=lo[:],
                        )

                # Transposed resident planes: [128 kpart, KT, BH]
                xt_hi = xtres.tile([P, KT, BH], bf16, name="xt_hi")
                nc.sync.dma_start_transpose(out=xt_hi[:], in_=xhi_nat[bp][:])
                xt_lo = xtres.tile([P, KT, BH], bf16, name="xt_lo")
                nc.sync.dma_start_transpose(out=xt_lo[:], in_=xlo_nat[bp][:])

                for s in range(NSLAB):
                    if bp == 0:
                        # ---- ternarize W rows of this slab: {-1,0,+1} bf16 ----
                        for jsub in range(JS // P):
                            j0 = s * JS + jsub * P
                            for kc in range(NKC):
                                c0 = kc * KCH
                                win = ws32.tile([P, KCH], f32, name="win")
                                nc.sync.dma_start(
                                    out=win[:], in_=w[j0 : j0 + P, c0 : c0 + KCH]
                                )
                                # a = (w > 1) in {0,1}
                                a = ws16.tile([P, KCH], bf16, name="wpos")
                                nc.vector.tensor_scalar(
                                    out=a[:], in0=win[:], scalar1=1.0, scalar2=None,
                                    op0=alu.is_gt,
                                )
                                # b2 = (w >= -1) - 1 in {-1,0}
                                b2 = ws16.tile([P, KCH], bf16, name="wneg")
                                nc.vector.tensor_scalar(
                                    out=b2[:], in0=win[:], scalar1=-1.0, scalar2=-1.0,
                                    op0=alu.is_ge, op1=alu.add,
                                )
                                t = ws16.tile([P, KCH], bf16, name="wtern")
                                nc.vector.tensor_add(out=t[:], in0=a[:], in1=b2[:])
                                nc.sync.dma_start(
                                    out=wt_nat[s][
                                        jsub * P : jsub * P + P, c0 : c0 + KCH
                                    ],
                                    in_=t[:],
                                )

                    # ---- transpose-load W'^T slab: [128 kpart, KT, JS] ----
                    wt = wtp.tile([P, KT, JS], bf16, name="wt")
                    nc.sync.dma_start_transpose(out=wt[:], in_=wt_nat[s][:])

                    # ---- matmuls: psum[b] += xt[:,k,b128].T @ wt[:,k,:] ----
                    psums = [
                        pp.tile([P, JS], f32, name=f"acc{b}") for b in range(BT)
                    ]
                    for k in range(KT):
                        for b in range(BT):
                            nc.tensor.matmul(
                                psums[b][:],
                                xt_hi[:, k, ts(b, P)],
                                wt[:, k, :],
                                start=(k == 0),
                                stop=False,
                            )
                            nc.tensor.matmul(
                                psums[b][:],
                                xt_lo[:, k, ts(b, P)],
                                wt[:, k, :],
                                start=False,
                                stop=(k == KT - 1),
                            )
                    # ---- threshold + store (natural [b, j] layout) ----
                    for b in range(BT):
                        spk = op.tile([P, JS], f32, name="spk")
                        nc.vector.tensor_scalar(
                            out=spk[:], in0=psums[b][:], scalar1=1.0, scalar2=None,
                            op0=alu.is_ge,
                        )
                        r0 = bp * BH + b * P
                        nc.sync.dma_start(
                            out=out[r0 : r0 + P, s * JS : s * JS + JS], in_=spk[:]
                        )

    nc.compile()
    return nc


def _get_built():
    global _BUILT
    if _BUILT is None:
        _BUILT = build_bass(B_CORE, IN_F, OUT_F)
    return _BUILT


def kernel(spike_input: np.ndarray, synapse_states: np.ndarray) -> np.ndarray:
    from concourse.bass_utils import run_bass_kernel_spmd

    nc = _get_built()
    xs = np.ascontiguousarray(spike_input, dtype=np.float32)
    ws = np.ascontiguousarray(synapse_states, dtype=np.float32)
    in_maps = [
        {"x": xs[c * B_CORE : (c + 1) * B_CORE], "w": ws} for c in range(N_CORES)
    ]
    res = run_bass_kernel_spmd(nc, in_maps, core_ids=list(range(N_CORES)))
    out = np.empty((BATCH, OUT_F), dtype=np.float32)
    for c in range(N_CORES):
        out[c * B_CORE : (c + 1) * B_CORE] = res.results[c]["out"]
    return out


# revision 5
# speedup vs baseline: 1.1954x; 1.1954x over previous
"""Trainium2 Bass kernel for LogicGatedSNN.

Computes: spikes = (spike_input @ ternarize(synapse_states).T >= 1.0)
  where ternarize(s) = +1 if s > 1, -1 if s < -1, else 0.

Strategy:
  - Data-parallel over the batch dim across 8 NeuronCores (1024 rows/core),
    weights replicated.
  - On each core: ternarize W on DVE into bf16 {-1,0,+1} (exact),
    split X into bf16 hi/lo planes (x = hi + lo, residual ~2^-18 * x),
    round-trip both through DRAM scratch and reload with xbar transpose-DMA
    to get the contraction dim (k) onto SBUF partitions.
  - Matmul: psum[b_tile 128, j 512] accumulated over k (32 steps x hi/lo),
    stationary = X^T plane tiles [128k, 128b], moving = W'^T [128k, 512j].
  - Threshold (>= 1.0) on DVE straight out of PSUM, store natural [b, j].
"""

import sys

if "/opt/trn_rl_repo" not in sys.path:
    sys.path.insert(0, "/opt/trn_rl_repo")

import numpy as np

N_CORES = 8
BATCH, IN_F, OUT_F = 8192, 4096, 4096
B_CORE = BATCH // N_CORES  # 1024

_BUILT = None


def build_bass(B, K, J, JS=512, KCH=1024, reps=1):
    """Build the per-core Bass program for x:[B,K] f32, w:[J,K] f32 -> out:[B,J] f32.

    reps > 1 repeats the whole compute (idempotent) for benchmarking via
    wall-clock deltas between builds with different reps.
    """
    from concourse import bacc
    import concourse.mybir as mybir
    import concourse.tile as tile
    from concourse.bass import ts

    f32, bf16 = mybir.dt.float32, mybir.dt.bfloat16
    alu = mybir.AluOpType
    P = 128
    JS = min(JS, J)
    KCH = min(KCH, K)
    BH = min(512, B)          # batch rows per pass (<= 4 psum banks)
    NBP = B // BH             # number of batch passes
    BT = BH // P              # 128-row tiles per pass
    KT = K // P               # k tiles (partition-dim groups)
    NSLAB = J // JS           # output-feature slabs
    NKC = K // KCH            # staging chunks along k
    assert B % BH == 0 and BH % P == 0 and K % P == 0 and J % JS == 0

    nc = bacc.Bacc("TRN2", target_bir_lowering=False, debug=False)
    x = nc.dram_tensor("x", [B, K], f32, kind="ExternalInput")
    w = nc.dram_tensor("w", [J, K], f32, kind="ExternalInput")
    out = nc.dram_tensor("out", [B, J], f32, kind="ExternalOutput")

    with tile.TileContext(nc) as tc:
        with (
            tc.tile_pool(name="dram", bufs=1, space="DRAM") as dpool,
            tc.tile_pool(name="xstage32", bufs=2) as xs32,
            tc.tile_pool(name="xstage16", bufs=2) as xs16,
            tc.tile_pool(name="wstage32", bufs=2) as ws32,
            tc.tile_pool(name="wstage16", bufs=2) as ws16,
            tc.tile_pool(name="xtres", bufs=1) as xtres,
            tc.tile_pool(name="wtp", bufs=2) as wtp,
            tc.tile_pool(name="ostage", bufs=8) as op,
            tc.tile_pool(name="psum", bufs=2, space="PSUM") as pp,
        ):
            # DRAM scratch: ternarized W (natural layout), per j-slab for
            # slab-granular RAW pipelining; X hi/lo planes per batch pass.
            wt_nat = [
                dpool.tile([JS, K], bf16, name=f"wt_nat_s{s}") for s in range(NSLAB)
            ]
            xhi_nat = [
                dpool.tile([BH, K], bf16, name=f"xhi_nat_p{bp}") for bp in range(NBP)
            ]
            xlo_nat = [
                dpool.tile([BH, K], bf16, name=f"xlo_nat_p{bp}") for bp in range(NBP)
            ]

            import itertools

            for rep, bp in itertools.product(range(reps), range(NBP)):
                # ---- X prep for this batch half: split f32 -> bf16 hi + lo ----
                for bsub in range(BT):
                    r0 = bp * BH + bsub * P
                    for kc in range(NKC):
                        c0 = kc * KCH
                        xin = xs32.tile([P, KCH], f32, name="xin")
                        nc.sync.dma_start(
                            out=xin[:], in_=x[r0 : r0 + P, c0 : c0 + KCH]
                        )
                        hi = xs16.tile([P, KCH], bf16, name="xhi")
                        nc.vector.tensor_copy(out=hi[:], in_=xin[:])
                        hi32 = xs32.tile([P, KCH], f32, name="xhi32")
                        nc.vector.tensor_copy(out=hi32[:], in_=hi[:])
                        lo = xs16.tile([P, KCH], bf16, name="xlo")
                        nc.vector.tensor_sub(out=lo[:], in0=xin[:], in1=hi32[:])
                        nc.sync.dma_start(
                            out=xhi_nat[bp][bsub * P : bsub * P + P, c0 : c0 + KCH],
                            in_=hi[:],
                        )
                        nc.sync.dma_start(
                            out=xlo_nat[bp][bsub * P : bsub * P + P, c0 : c0 + KCH],
                            in_=lo[:],
                        )

                # Transposed resident planes: [128 kpart, KT, BH]
                xt_hi = xtres.tile([P, KT, BH], bf16, name="xt_hi")
                nc.sync.dma_start_transpose(out=xt_hi[:], in_=xhi_nat[bp][:])
                xt_lo = xtres.tile([P, KT, BH], bf16, name="xt_lo")
                nc.sync.dma_start_transpose(out=xt_lo[:], in_=xlo_nat[bp][:])

                for s in range(NSLAB):
                    if bp == 0:
                        # ---- ternarize W rows of this slab: {-1,0,+1} bf16 ----
                        for jsub in range(JS // P):
                            j0 = s * JS + jsub * P
                            for kc in range(NKC):
                                c0 = kc * KCH
                                win = ws32.tile([P, KCH], f32, name="win")
                                nc.sync.dma_start(
                                    out=win[:], in_=w[j0 : j0 + P, c0 : c0 + KCH]
                                )
                                # a = (w > 1) in {0,1}
                                a = ws16.tile([P, KCH], bf16, name="wpos")
                                nc.vector.tensor_scalar(
                                    out=a[:], in0=win[:], scalar1=1.0, scalar2=None,
                                    op0=alu.is_gt,
                                )
                                # b2 = (w >= -1) - 1 in {-1,0}
                                b2 = ws16.tile([P, KCH], bf16, name="wneg")
                                nc.vector.tensor_scalar(
                                    out=b2[:], in0=win[:], scalar1=-1.0, scalar2=-1.0,
                                    op0=alu.is_ge, op1=alu.add,
                                )
                                t = ws16.tile([P, KCH], bf16, name="wtern")
                                nc.vector.tensor_add(out=t[:], in0=a[:], in1=b2[:])
                                nc.sync.dma_start(
                                    out=wt_nat[s][
                                        jsub * P : jsub * P + P, c0 : c0 + KCH
                                    ],
                                    in_=t[:],
                                )

                    # ---- transpose-load W'^T slab: [128 kpart, KT, JS] ----
                    wt = wtp.tile([P, KT, JS], bf16, name="wt")
                    nc.sync.dma_start_transpose(out=wt[:], in_=wt_nat[s][:])

                    # ---- matmuls: psum[b] += xt[:,k,b128].T @ wt[:,k,:] ----
                    psums = [
                        pp.tile([P, JS], f32, name=f"acc{b}") for b in range(BT)
                    ]
                    for k in range(KT):
                        for b in range(BT):
                            nc.tensor.matmul(
                                psums[b][:],
                                xt_hi[:, k, ts(b, P)],
                                wt[:, k, :],
                                start=(k == 0),
                                stop=False,
                            )
                            nc.tensor.matmul(
                                psums[b][:],
                                xt_lo[:, k, ts(b, P)],
                                wt[:, k, :],
                                start=False,
                                stop=(k == KT - 1),
                            )
                    # ---- threshold + store (natural [b, j] layout) ----
                    for b in range(BT):
                        spk = op.tile([P, JS], f32, name="spk")
                        nc.vector.tensor_scalar(
                            out=spk[:], in0=psums[b][:], scalar1=1.0, scalar2=None,
                            op0=alu.is_ge,
                        )
                        r0 = bp * BH + b * P
                        nc.sync.dma_start(
                            out=out[r0 : r0 + P, s * JS : s * JS + JS], in_=spk[:]
                        )

    nc.compile()
    return nc


def _get_built():
    global _BUILT
    if _BUILT is None:
        _BUILT = build_bass(B_CORE, IN_F, OUT_F)
    return _BUILT


def kernel(spike_input: np.ndarray, synapse_states: np.ndarray) -> np.ndarray:
    from concourse.bass_utils import run_bass_kernel_spmd

    nc = _get_built()
    xs = np.ascontiguousarray(spike_input, dtype=np.float32)
    ws = np.ascontiguousarray(synapse_states, dtype=np.float32)
    in_maps = [
        {"x": xs[c * B_CORE : (c + 1) * B_CORE], "w": ws} for c in range(N_CORES)
    ]
    res = run_bass_kernel_spmd(nc, in_maps, core_ids=list(range(N_CORES)))
    out = np.empty((BATCH, OUT_F), dtype=np.float32)
    for c in range(N_CORES):
        out[c * B_CORE : (c + 1) * B_CORE] = res.results[c]["out"]
    return out
